# revision 1
# baseline (speedup 1.0000x reference)
"""Fused LayerNorm + multi-head attention + output projection on 8 TRN2 NeuronCores.

Problem (hardcoded shapes): x [2, 2048, 1024] f32, 16 heads x 64 dim.
Sharding: data-parallel over batch (2) x tensor-parallel over head groups (4).
Core c handles batch c//4, heads [4*(c%4), 4*(c%4)+4). W_qkv column-sharded,
W_out row-sharded; per-core partial outputs are summed on the host.

v2 layout strategy (per core):
  - Host passes BOTH x [tok, dim] bf16 (for LN stats) and xT [dim, tok] bf16
    (pre-transposed on host) -- the on-device xn transpose pass is gone.
  - LN is folded into the projections: q/k/v are computed from RAW xT with a
    K=1 "appendix" matmul adding -mu[t]*colsum(W) (exact centering), then the
    per-token rs=1/sqrt(var+eps) scale is applied during PSUM evacuation:
    q/k multiply by a broadcast RS tile (built by a K=1 ones matmul from an
    rs row obtained with tiny single-column PE transposes); v multiplies by
    rs as a per-partition scalar. k's LN beta is dropped entirely (softmax
    shift invariance); v's beta rides the attnT evacuation (sum a = 1).
  - rs = exp(-0.5*ln(var+eps)) on ACT: stays in the exp table set (no
    sqrt-set thrash).
  - q,k are produced transposed (qT/kT [dhead, tok]) and DUPLICATED into both
    partition halves, so scores for two consecutive k-chunks run as two
    concurrent K=64 matmuls in different PE row-groups.
  - scoresT [ktok, q]: softmax exp on ACT (PSUM -> SBUF bf16, matmul scale
    folded), except every 8th cell which uses a Schraudolph bit-trick fast
    exp on DVE (tensor_scalar to int32, bitcast copy) to offload ACT.
  - AV keeps V stationary ([v|ones] 65-col weights) and streams expT; outT
    [65, q] accumulates in PSUM with softmax denominators in row 64; cheap
    per-q-tile PE back-transposes + reciprocal scale the output.
  - A warm-up burst of dummy matmuls runs during the initial x DMA so the
    PE HAM clock-gate reaches 8/8 before real work.
  - Output projection keeps W_out stationary, emitting partial [dim, tok]
    in bf16 (summed in fp32 on host).
"""

import numpy as np
import ml_dtypes

B, N, DIM = 2, 2048, 1024
HEADS, DIM_HEAD = 16, 64
INNER = HEADS * DIM_HEAD
NCORES = 8
HG = 4                      # head-groups
HL = HEADS // HG            # heads per core (local)
QT = N // 128               # 16 q-tiles of 128 tokens
KC = N // 128               # 16 k-chunks of 128 tokens
KP = KC // 2                # 8 k-chunk pairs
DC = DIM // 128             # 8 dim chunks
TC4 = N // 512              # 4 chunks of 512 tokens
SCALE = DIM_HEAD ** -0.5
EPS = 1e-5

# Schraudolph fast-exp constants: exp(SCALE*s) ~ bitcast(int32(A*s + B))
SCHRA_A = float((1 << 23) * 1.4426950408889634 * SCALE)
SCHRA_B = float((127 << 23) - 486411)
FAST_EVERY = 12             # every 12th attention cell uses DVE fast-exp

_cache = {}


def _build():
    import concourse.bass as bass
    import concourse.tile as tile
    from concourse import bacc, mybir

    f32 = mybir.dt.float32
    bf16 = mybir.dt.bfloat16
    i32 = mybir.dt.int32
    AF = mybir.ActivationFunctionType
    ALU = mybir.AluOpType

    nc = bacc.Bacc("TRN2", target_bir_lowering=False, debug=False,
                   num_devices=NCORES)

    x_d = nc.dram_tensor("x", [N, DIM], bf16, kind="ExternalInput").ap()
    xt_d = nc.dram_tensor("xt", [DC, 128, N], bf16, kind="ExternalInput").ap()
    wqk_d = nc.dram_tensor("wqk", [128, DC * 512], bf16,
                           kind="ExternalInput").ap()
    wv_d = nc.dram_tensor("wv", [128, DC * 256], bf16,
                          kind="ExternalInput").ap()
    wqkcs_d = nc.dram_tensor("wqkcs", [1, 512], bf16,
                             kind="ExternalInput").ap()
    wvcs_d = nc.dram_tensor("wvcs", [1, 256], bf16,
                            kind="ExternalInput").ap()
    wout_d = nc.dram_tensor("wout", [128, 2 * DIM], bf16,
                            kind="ExternalInput").ap()
    bq_d = nc.dram_tensor("bq", [128, 2], f32, kind="ExternalInput").ap()
    bv_d = nc.dram_tensor("bv", [128, 2], f32, kind="ExternalInput").ap()
    ones_d = nc.dram_tensor("ones", [1, 128], bf16, kind="ExternalInput").ap()
    ident_d = nc.dram_tensor("ident", [128, 128], bf16,
                             kind="ExternalInput").ap()
    identf_d = nc.dram_tensor("identf", [128, 128], f32,
                              kind="ExternalInput").ap()
    out_d = nc.dram_tensor("out", [DIM, N], bf16, kind="ExternalOutput").ap()

    with tile.TileContext(nc) as tc:
        _graph(nc, tc, tile, bass, mybir, f32, bf16, i32, AF, ALU,
               x_d, xt_d, wqk_d, wv_d, wqkcs_d, wvcs_d, wout_d,
               bq_d, bv_d, ones_d, ident_d, identf_d, out_d)
    nc.compile()
    return nc


def _graph(nc, tc, tile, bass, mybir, f32, bf16, i32, AF, ALU,
           x_d, xt_d, wqk_d, wv_d, wqkcs_d, wvcs_d, wout_d,
           bq_d, bv_d, ones_d, ident_d, identf_d, out_d):
    from collections import deque
    from contextlib import ExitStack
    ctx = ExitStack()
    with ctx:
        # ---- persistent SBUF tensors -------------------------------------
        pers = ctx.enter_context(tc.tile_pool(name="pers", bufs=1))
        xT = [pers.tile([128, N], bf16, tag=f"xT{d}", name=f"xT{d}")
              for d in range(DC)]                                # 4 MB
        # duplicated-transposed q and k: [h][tchunk] -> [128, 512]
        q2 = [[pers.tile([128, 512], bf16, tag=f"q2_{h}_{t}",
                         name=f"q2_{h}_{t}") for t in range(TC4)]
              for h in range(HL)]
        k2 = [[pers.tile([128, 512], bf16, tag=f"k2_{h}_{t}",
                         name=f"k2_{h}_{t}") for t in range(TC4)]
              for h in range(HL)]
        v_ones = [pers.tile([128, 4, HL, DIM_HEAD + 1], bf16,
                            tag=f"vo{t}", name=f"vo{t}")
                  for t in range(TC4)]
        attn_s = pers.tile([128, QT, HL * DIM_HEAD], bf16, tag="attn")
        attnT = pers.tile([128, 2, N], bf16, tag="attnT")        # 1 MB
        wqk = pers.tile([128, DC, 512], bf16, tag="wqk")         # 1 MB
        wv = pers.tile([128, DC, 256], bf16, tag="wv")
        wout = pers.tile([128, 2, DIM], bf16, tag="wout")
        wqkcs = pers.tile([1, 512], bf16, tag="wqkcs")
        wvcs = pers.tile([1, 256], bf16, tag="wvcs")
        bq = pers.tile([128, 2], f32, tag="bq")
        bv = pers.tile([128, 2], f32, tag="bv")
        ones_c = pers.tile([1, 128], bf16, tag="ones")
        ident = pers.tile([128, 128], bf16, tag="ident")
        ident_f = pers.tile([128, 128], f32, tag="identf")
        rs_all = pers.tile([128, QT], f32, tag="rs_all")         # rs columns
        nmu_all = pers.tile([128, QT], f32, tag="nmu_all")       # -mu columns
        rows_sb = [pers.tile([1, 1024], bf16, tag=f"rows{t}",
                             name=f"rows{t}") for t in range(TC4)]

        sb_x0 = ctx.enter_context(tc.tile_pool(name="sb_x0", bufs=16))
        ps_o = ctx.enter_context(tc.tile_pool(name="ps_o", bufs=4,
                                              space="PSUM"))

        # tiny consts first on the DMA queue: the row transposes / bcasts /
        # evacuations need them and the PE queue is in-order.
        nc.sync.dma_start(ident[:], ident_d[:])
        nc.sync.dma_start(ident_f[:], identf_d[:])
        nc.sync.dma_start(ones_c[:], ones_d[:])
        nc.sync.dma_start(wqkcs[:], wqkcs_d[:])
        nc.sync.dma_start(wvcs[:], wvcs_d[:])
        nc.sync.dma_start(bq[:], bq_d[:])
        nc.sync.dma_start(bv[:], bv_d[:])

        # warm-up: dummy matmuls on ident keep PE busy during initial DMA so
        # the HAM clock-gate reaches 8/8 before real work
        pw = ps_o.tile([128, 512], f32, tag="O", name="pw")
        for i in range(56):
            nc.tensor.matmul(pw[:, 0:128], ident[:], ident[:],
                             start=True, stop=True)

        xpre = [sb_x0.tile([128, DIM], bf16, tag="x", name=f"xpre{t}")
                for t in range(QT)]

        def dma_x(t):
            nc.sync.dma_start(xpre[t][:], x_d[t * 128:(t + 1) * 128, :])

        def dma_xT(d):
            nc.scalar.dma_start(xT[d][:], xt_d[d])

        # big weight/xT loads ride the Activation engine's HWDGE queue (ACT
        # is idle during the load phase) so they run parallel to the x loads
        # on the SP queue; whole-tile transfers keep 4-8KB partition lines.
        nc.scalar.dma_start(wqk[:].rearrange("p a b -> p (a b)"), wqk_d[:])
        for d in range(DC):
            dma_xT(d)
        nc.scalar.dma_start(wv[:].rearrange("p a b -> p (a b)"), wv_d[:])
        nc.scalar.dma_start(wout[:].rearrange("p a b -> p (a b)"), wout_d[:])
        for t in range(QT):
            dma_x(t)
        for t in range(TC4):
            nc.gpsimd.memset(v_ones[t][:], 1.0)

        # ---- pools -------------------------------------------------------
        sb_x = ctx.enter_context(tc.tile_pool(name="sb_x", bufs=5))
        sb_st = ctx.enter_context(tc.tile_pool(name="sb_st", bufs=12))
        sb_rs = ctx.enter_context(tc.tile_pool(name="sb_rs", bufs=2))
        sb_exp = ctx.enter_context(tc.tile_pool(name="sb_exp", bufs=8))
        sb_ei = ctx.enter_context(tc.tile_pool(name="sb_ei", bufs=2))
        sb_u = ctx.enter_context(tc.tile_pool(name="sb_u", bufs=2))
        sb_o = ctx.enter_context(tc.tile_pool(name="sb_o", bufs=4))
        sb_qk = ctx.enter_context(tc.tile_pool(name="sb_qk", bufs=3))
        ps_a = ctx.enter_context(tc.tile_pool(name="ps_a", bufs=2,
                                              space="PSUM"))

        # ---- phase 2 machinery (same schedule as baseline) ---------------
        deferred = deque()
        po_q = {}
        cellctr = [0]

        def emit_av(h, jp, qc, et):
            if (h, qc) not in po_q:
                po_q[(h, qc)] = ps_o.tile([65, 512], f32, tag="O",
                                          name=f"po{h}_{qc}")
            po = po_q[(h, qc)]
            for i in range(2):
                k = 2 * jp + i
                nc.tensor.matmul(po[:],
                                 v_ones[k // 4][:, k % 4, h, :],
                                 et[:, i * 512:(i + 1) * 512],
                                 start=(jp == 0 and i == 0),
                                 stop=(jp == KP - 1 and i == 1))

        def emit_drain(h):
            ou = sb_u.tile([65, N], bf16, tag="u")
            for qc in range(4):
                nc.vector.tensor_copy(ou[:, qc * 512:(qc + 1) * 512],
                                      po_q.pop((h, qc))[:])
            for half in range(2):
                pbt = ps_o.tile([128, 8, 128], bf16, tag="O",
                                name=f"pbt{h}_{half}")
                for j in range(8):
                    qt = half * 8 + j
                    nc.tensor.transpose(pbt[:, j, 0:65],
                                        ou[:, qt * 128:(qt + 1) * 128],
                                        ident[0:65, 0:65])
                for j in range(8):
                    qt = half * 8 + j
                    rec = sb_st.tile([128, 1], f32, tag="rec")
                    nc.vector.reciprocal(rec[:], pbt[:, j, 64:65])
                    nc.vector.tensor_scalar_mul(
                        attn_s[:, qt, h * DIM_HEAD:(h + 1) * DIM_HEAD],
                        pbt[:, j, 0:DIM_HEAD], rec[:])

        def emit_tail3(qc):
            h = HL - 1
            oup = sb_u.tile([65, 512], bf16, tag="up")
            nc.vector.tensor_copy(oup[:], po_q.pop((h, qc))[:])
            pbt = ps_o.tile([128, 4, 128], bf16, tag="O", name=f"pbt3_{qc}")
            for j in range(4):
                nc.tensor.transpose(pbt[:, j, 0:65],
                                    oup[:, j * 128:(j + 1) * 128],
                                    ident[0:65, 0:65])
            for j in range(4):
                qt = qc * 4 + j
                rec = sb_st.tile([128, 1], f32, tag="rec")
                nc.vector.reciprocal(rec[:], pbt[:, j, 64:65])
                nc.vector.tensor_scalar_mul(
                    attn_s[:, qt, h * DIM_HEAD:(h + 1) * DIM_HEAD],
                    pbt[:, j, 0:DIM_HEAD], rec[:])
            pt2 = ps_o.tile([128, 4, 256], bf16, tag="O", name=f"pt2_{qc}")
            for j in range(4):
                qt = qc * 4 + j
                for i in range(2):
                    nc.tensor.transpose(
                        pt2[:, j, i * 128:(i + 1) * 128],
                        attn_s[:, qt, i * 128:(i + 1) * 128], ident[:])
            for j in range(4):
                qt = qc * 4 + j
                for i in range(2):
                    nc.vector.tensor_scalar_add(
                        attnT[:, i, qt * 128:(qt + 1) * 128],
                        pt2[:, j, i * 128:(i + 1) * 128], bv[:, i:i + 1])

        def flush_one():
            task = deferred.popleft()
            if task[0] == "av":
                emit_av(task[1], task[2], task[3], task[4])
            elif task[0] == "tail3":
                emit_tail3(task[1])
            elif task[0] == "drainh":
                emit_drain_half(task[1], task[2])
            else:
                emit_drain(task[1])

        def emit_cell(h, jp, qc):
            ke, ko = 2 * jp, 2 * jp + 1
            tcq = ke // 4
            fast = (cellctr[0] % FAST_EVERY == FAST_EVERY - 1)
            cellctr[0] += 1
            pscr = ps_a.tile([128, 1024], f32, tag="A")
            nc.tensor.matmul(
                pscr[:, 0:512],
                k2[h][tcq][0:64, (ke % 4) * 128:(ke % 4) * 128 + 128],
                q2[h][qc][0:64, :],
                start=True, stop=True)
            nc.tensor.matmul(
                pscr[:, 512:1024],
                k2[h][tcq][64:128, (ko % 4) * 128:(ko % 4) * 128 + 128],
                q2[h][qc][64:128, :],
                start=True, stop=True)
            et = sb_exp.tile([128, 1024], bf16, tag="e")
            if fast:
                ei = sb_ei.tile([128, 1024], i32, tag="ei")
                nc.vector.tensor_scalar(ei[:], pscr[:], SCHRA_A, SCHRA_B,
                                        op0=ALU.mult, op1=ALU.add)
                nc.vector.tensor_copy(et[:], ei[:].bitcast(f32))
            else:
                nc.scalar.activation(et[:], pscr[:], AF.Exp, scale=SCALE)
            deferred.append(("av", h, jp, qc, et))
            while len(deferred) > 4:
                flush_one()

        def emit_drain_half(h, half):
            ouh = sb_u.tile([65, N // 2], bf16, tag="uh", name=f"ouh{h}_{half}")
            for i in range(2):
                qc = half * 2 + i
                nc.vector.tensor_copy(ouh[:, i * 512:(i + 1) * 512],
                                      po_q.pop((h, qc))[:])
            pbt = ps_o.tile([128, 8, 128], bf16, tag="O",
                            name=f"pbth{h}_{half}")
            for j in range(8):
                nc.tensor.transpose(pbt[:, j, 0:65],
                                    ouh[:, j * 128:(j + 1) * 128],
                                    ident[0:65, 0:65])
            for j in range(8):
                qt = half * 8 + j
                rec = sb_st.tile([128, 1], f32, tag="rec")
                nc.vector.reciprocal(rec[:], pbt[:, j, 64:65])
                nc.vector.tensor_scalar_mul(
                    attn_s[:, qt, h * DIM_HEAD:(h + 1) * DIM_HEAD],
                    pbt[:, j, 0:DIM_HEAD], rec[:])

        def wavefront_cells(tci):
            cells = []
            for jp in (tci * 2, tci * 2 + 1):
                for qc in range(tci + 1):
                    cells.append((jp, qc))
            for jp in range(tci * 2):
                cells.append((jp, tci))
            cells.sort(key=lambda c: (c[1], c[0]))
            return cells

        def _emit_v(tchunk, rows):
            for j in range(4):
                t = tchunk * 4 + j
                pv = ps_a.tile([128, 256], f32, tag="A", name=f"pv{tchunk}_{j}")
                for d in range(DC):
                    nc.tensor.matmul(pv[:],
                                     xT[d][:, t * 128:(t + 1) * 128],
                                     wv[:, d, :], start=(d == 0),
                                     stop=False)
                nc.tensor.matmul(pv[:], rows[0:1, j * 128:(j + 1) * 128],
                                 wvcs[0:1, :], start=False, stop=True)
                nc.vector.tensor_scalar_mul(
                    v_ones[tchunk][:, j, :, 0:DIM_HEAD],
                    pv[:].rearrange("p (a b) -> p a b", a=HL),
                    rs_all[:, t:t + 1])

        # ---- phase 1: stats, rows, q2/k2, v (per 512-token chunk) --------
        veps_all = pers.tile([128, QT], f32, tag="veps_all")
        for tchunk in range(TC4):
            for j in range(4):
                t = tchunk * 4 + j
                xt = xpre[t]
                st6 = sb_st.tile([128, 2, 6], f32, tag="st6")
                nc.vector.bn_stats(st6[:, 0, :], xt[:, 0:512])
                nc.vector.bn_stats(st6[:, 1, :], xt[:, 512:1024])
                mv = sb_st.tile([128, 2], f32, tag="mv")
                nc.vector.bn_aggr(mv[:], st6[:].rearrange("p a b -> p (a b)"))
                nc.vector.tensor_scalar_add(veps_all[:, t:t + 1],
                                            mv[:, 1:2], EPS)
                nc.vector.tensor_scalar_mul(nmu_all[:, t:t + 1], mv[:, 0:1],
                                            -1.0)
            # batched quake rsqrt (+1 Newton) for the 4 tiles of this chunk
            tsl = slice(tchunk * 4, tchunk * 4 + 4)
            y0 = sb_st.tile([128, 4], i32, tag="y0")
            nc.vector.tensor_scalar(y0[:], veps_all[:, tsl].bitcast(i32),
                                    -0.5, float(0x5f3759df),
                                    op0=ALU.mult, op1=ALU.add)
            t1 = sb_st.tile([128, 4], f32, tag="t1")
            nc.vector.tensor_mul(t1[:], y0[:].bitcast(f32),
                                 y0[:].bitcast(f32))
            nc.vector.tensor_mul(t1[:], t1[:], veps_all[:, tsl])
            nc.vector.tensor_scalar(t1[:], t1[:], -0.5, 1.5,
                                    op0=ALU.mult, op1=ALU.add)
            nc.vector.tensor_mul(rs_all[:, tsl], t1[:], y0[:].bitcast(f32))
            # burst: columns -> rows (short-lived PSUM tile)
            rows_ps = ps_a.tile([1, 1024], f32, tag="A",
                                name=f"rowsps{tchunk}")
            for j in range(4):
                t = tchunk * 4 + j
                nc.tensor.transpose(rows_ps[0:1, j * 128:(j + 1) * 128],
                                    nmu_all[:, t:t + 1], ident_f[:])
                nc.tensor.transpose(
                    rows_ps[0:1, 512 + j * 128:512 + (j + 1) * 128],
                    rs_all[:, t:t + 1], ident_f[:])
            rows = rows_sb[tchunk]
            nc.vector.tensor_copy(rows[0:1, :], rows_ps[0:1, :])
            # broadcast rs row -> RS [128, 512]
            rs_ps = ps_a.tile([128, 512], f32, tag="A", name=f"rsps{tchunk}")
            nc.tensor.matmul(rs_ps[:], ones_c[0:1, :], rows[0:1, 512:1024],
                             start=True, stop=True)
            RS = sb_rs.tile([128, 512], bf16, tag="RS")
            nc.vector.tensor_copy(RS[:], rs_ps[:])
            # q/k (transposed, duplicated halves) for this token chunk
            for c in range(4):
                pq = ps_a.tile([128, 512], f32, tag="A")
                for d in range(DC):
                    nc.tensor.matmul(pq[:], wqk[:, d, c * 128:(c + 1) * 128],
                                     xT[d][:, tchunk * 512:(tchunk + 1) * 512],
                                     start=(d == 0), stop=False)
                nc.tensor.matmul(pq[:], wqkcs[0:1, c * 128:(c + 1) * 128],
                                 rows[0:1, 0:512], start=False, stop=True)
                dst = q2 if c < 2 else k2
                hA, hB = (c % 2) * 2, (c % 2) * 2 + 1
                qktmp = sb_qk.tile([128, 512], bf16, tag="qktmp")
                nc.vector.tensor_mul(qktmp[:], pq[:], RS[:])
                if c < 2:
                    nc.vector.tensor_scalar_add(qktmp[:], qktmp[:],
                                                bq[:, c:c + 1])
                for hh, rows_sl in ((hA, slice(0, 64)), (hB, slice(64, 128))):
                    for half in range(2):
                        nc.sync.dma_start(
                            dst[hh][tchunk][half * 64:half * 64 + 64, :],
                            qktmp[rows_sl, :])
            _emit_v(tchunk, rows)
            if tchunk > 0:
                for jp, qc in wavefront_cells(tchunk - 1):
                    if qc <= 1:
                        emit_cell(0, jp, qc)
                        emit_cell(1, jp, qc)

        # ---- phase 2 tail: finish head 0, then heads 1-3 -----------------
        for jp, qc in wavefront_cells(TC4 - 1):
            if qc <= 1:
                emit_cell(0, jp, qc)
                emit_cell(1, jp, qc)
        deferred.append(("drainh", 0, 0))
        deferred.append(("drainh", 1, 0))
        for h in (0, 1):
            for qc in (2, 3):
                for jp in range(KP):
                    emit_cell(h, jp, qc)
            deferred.append(("drainh", h, 1))
        for jp in range(KP):
            for qc in range(4):
                emit_cell(2, jp, qc)
        deferred.append(("drain", 2))
        for qc in range(4):
            for jp in range(KP):
                emit_cell(HL - 1, jp, qc)
            deferred.append(("tail3", qc))
        while deferred:
            flush_one()

        # ---- phase 3: output projection ----------------------------------
        for tp in range(TC4):
            for dcc in range(DC):
                po2 = ps_o.tile([128, 512], f32, tag="O",
                                name=f"po2_{tp}_{dcc}")
                for i in range(2):
                    nc.tensor.matmul(po2[:],
                                     wout[:, i, dcc * 128:(dcc + 1) * 128],
                                     attnT[:, i, tp * 512:(tp + 1) * 512],
                                     start=(i == 0), stop=(i == 1))
                ot = sb_o.tile([128, 512], bf16, tag="o")
                if dcc % 2 == 0:
                    nc.vector.tensor_copy(ot[:], po2[:])
                else:
                    nc.scalar.copy(ot[:], po2[:])
                nc.sync.dma_start(
                    out_d[dcc * 128:(dcc + 1) * 128, tp * 512:(tp + 1) * 512],
                    ot[:])


def _host_inputs(x, ln_gamma, ln_beta, W_qkv):
    """Per-core input maps (weights gamma-folded, bf16, head-group sharded)."""
    Wg = (ln_gamma[:, None] * W_qkv).astype(np.float32)
    beta_full = (ln_beta @ W_qkv).astype(np.float32)
    colsum = Wg.sum(axis=0).astype(np.float32)    # [3*INNER]
    in_maps = []
    for c in range(NCORES):
        b, hg = c // HG, c % HG
        qcols = slice(256 * hg, 256 * hg + 256)
        kcols = slice(INNER + 256 * hg, INNER + 256 * hg + 256)
        vcols = slice(2 * INNER + 256 * hg, 2 * INNER + 256 * hg + 256)
        wqk = np.concatenate([Wg[:, qcols], Wg[:, kcols]], axis=1)
        wvv = Wg[:, vcols]
        wqkcs = np.concatenate([colsum[qcols], colsum[kcols]])
        wvcs = colsum[vcols]
        bqq = beta_full[qcols]
        bvv = beta_full[vcols]
        xb = np.ascontiguousarray(x[b]).astype(ml_dtypes.bfloat16)
        xtb = np.ascontiguousarray(x[b].T).astype(ml_dtypes.bfloat16)
        in_maps.append({
            "x": xb,
            "xt": np.ascontiguousarray(xtb.reshape(DC, 128, N)),
            "wqk": np.ascontiguousarray(
                wqk.reshape(DC, 128, 512).transpose(1, 0, 2).reshape(
                    128, DC * 512)).astype(ml_dtypes.bfloat16),
            "wv": np.ascontiguousarray(
                wvv.reshape(DC, 128, 256).transpose(1, 0, 2).reshape(
                    128, DC * 256)).astype(ml_dtypes.bfloat16),
            "wqkcs": np.ascontiguousarray(
                wqkcs.reshape(1, 512)).astype(ml_dtypes.bfloat16),
            "wvcs": np.ascontiguousarray(
                wvcs.reshape(1, 256)).astype(ml_dtypes.bfloat16),
            "wout": None,  # filled by caller (needs W_out)
            "bq": np.ascontiguousarray(
                bqq.reshape(2, 128).T).astype(np.float32),
            "bv": np.ascontiguousarray(
                bvv.reshape(2, 128).T).astype(np.float32),
            "ones": np.ones((1, 128), dtype=np.float32).astype(
                ml_dtypes.bfloat16),
            "ident": np.eye(128, dtype=np.float32).astype(ml_dtypes.bfloat16),
            "identf": np.eye(128, dtype=np.float32),
        })
    return in_maps


def kernel(x, ln_gamma, ln_beta, W_qkv, W_out, b_out):
    from concourse.bass_utils import run_bass_kernel_spmd

    if "nc" not in _cache:
        _cache["nc"] = _build()
    nc = _cache["nc"]

    x = np.asarray(x, dtype=np.float32)
    ln_gamma = np.asarray(ln_gamma, dtype=np.float32)
    ln_beta = np.asarray(ln_beta, dtype=np.float32)
    W_qkv = np.asarray(W_qkv, dtype=np.float32)
    W_out = np.asarray(W_out, dtype=np.float32)
    b_out = np.asarray(b_out, dtype=np.float32)

    in_maps = _host_inputs(x, ln_gamma, ln_beta, W_qkv)
    for c in range(NCORES):
        hg = c % HG
        wo = W_out[256 * hg:256 * hg + 256, :]
        in_maps[c]["wout"] = np.ascontiguousarray(
            wo.reshape(2, 128, DIM).transpose(1, 0, 2).reshape(
                128, 2 * DIM)).astype(ml_dtypes.bfloat16)

    res = run_bass_kernel_spmd(nc, in_maps, core_ids=list(range(NCORES)))
    kernel._last_results = res

    out = np.empty((B, N, DIM), dtype=np.float32)
    for b in range(B):
        acc = np.zeros((DIM, N), dtype=np.float32)
        for hg in range(HG):
            acc += res.results[b * HG + hg]["out"].astype(np.float32)
        out[b] = acc.T + b_out
    return out

